# revision 10
# baseline (speedup 1.0000x reference)
"""GPT2 attention (B=4, S=2048, D=768, H=12, no causal mask) on 8 trn2 cores.

Sharding: core c -> batch b = c//2, head-group g = c%2 (6 heads of 64).
Each core computes its 6 heads' attention + the matching row-block of the
output projection; host sums the two per-batch partials and adds b_proj.

Host-side prep: x is pre-transposed to xT [768, 2048] (so no PE transposes
on device) and the 1/sqrt(hd) scale is folded into W_q / b_q.

On-chip layout (per core):
  xT    [768, 2048] bf16 (6 tiles [128, S]) -- loaded directly
  qT/kT [384, 2048] (3 tiles [128, S], one head pair per tile)
  vaug  [128, 6, 16, 128] -- v natural [sk, hd] per (head, sk-chunk),
        cols 64:128 = ones (denominator rows of the AV accumulation)
  scores per (pair, sq-block, sk-chunk): one psum tile [128 sk, 1024]
        (head A in cols 0:512, head B in 512:1024; 2 banks), exp'd by a
        single ACT instruction into SBUF bf16, then two AV accumulation
        matmuls with vaug (rows 64:127 of acc = softmax denominator).
  Normalize: DVE reciprocal + DVE multiply into attnT (natural lhsT for
  the projection).  proj: out [S, 768] partial to DRAM per sq tile.

The attention phase is ACT(exp)-bound (~1.1us per [128,1024] exp); qkv/proj
matmuls are emitted after the attention steps of each block so the
scheduler slots them into PE idle time under the exp stream.
"""

import json
from contextlib import ExitStack

import ml_dtypes
import numpy as np

import concourse.bass as bass
import concourse.mybir as mybir
import concourse.tile as tile
from concourse import library_config
from concourse.bass_utils import run_bass_kernel_spmd

B, S, D = 4, 2048, 768
H, HD = 12, 64
HPC = 6            # heads per core
DKC = HPC * HD     # 384: per-core width of q/k/v
NPAIR = HPC // 2   # 3 head pairs
P = 128
F32 = mybir.dt.float32
BF16 = mybir.dt.bfloat16

NSQ = S // 512     # 4 sq blocks
NST = S // 128     # 16 s tiles
NDC = D // 128     # 6 d chunks


def _split_multi_waits(bir_bytes):
    """Walrus in this toolchain accepts only one sync-wait per instruction.

    Hoist extra waits onto same-engine NoOps inserted just before. Engines
    execute their stream in order and semaphores are monotonic, so this is
    semantically identical.
    """
    m = json.loads(bir_bytes)
    n_split = 0
    for fn in m["functions"]:
        for blk in fn["blocks"]:
            new = []
            for ins in blk["instructions"]:
                si = ins.get("sync_info")
                waits = (si or {}).get("on_wait") or []
                if len(waits) > 1:
                    n_split += 1
                    for j, w in enumerate(waits[:-1]):
                        new.append({
                            "debug": ins.get("debug", 0),
                            "engine": ins["engine"],
                            "ins": [], "outs": [],
                            "name": f"{ins['name']}w{j}",
                            "opcode": "NoOp",
                            "sync_info": {"on_update": [], "on_wait": [w]},
                        })
                    si["on_wait"] = [waits[-1]]
                new.append(ins)
            blk["instructions"] = new
    return json.dumps(m).encode()


def build_kernel():
    nc = bass.Bass("TRN2", target_bir_lowering=False, debug=False)
    xT_d = nc.dram_tensor("xT", [D, S], BF16, kind="ExternalInput").ap()
    wqkv_d = nc.dram_tensor("wqkv", [D, 3 * DKC], BF16, kind="ExternalInput").ap()
    bqkv_d = nc.dram_tensor("bqkv", [3 * DKC], F32, kind="ExternalInput").ap()
    wproj_d = nc.dram_tensor("wproj", [DKC, D], BF16, kind="ExternalInput").ap()
    out_d = nc.dram_tensor("out", [S, D], F32, kind="ExternalOutput").ap()

    with tile.TileContext(nc) as tc:
        with ExitStack() as ctx:
            _body(ctx, tc, xT_d, wqkv_d, bqkv_d, wproj_d, out_d)
    orig_to_json = nc.to_json_bytes
    nc.to_json_bytes = lambda: _split_multi_waits(orig_to_json())
    return nc


def _body(ctx, tc, xT_d, wqkv_d, bqkv_d, wproj_d, out_d):
    nc = tc.nc
    ADD = mybir.AluOpType.add
    MULT = mybir.AluOpType.mult
    EXP = mybir.ActivationFunctionType.Exp

    consts = ctx.enter_context(tc.tile_pool(name="consts", bufs=1))
    big = ctx.enter_context(tc.tile_pool(name="big", bufs=1))
    # PSUM: exactly 8 banks: scores 2x[128,1024] + acc 2x[128,512] + misc 2
    scP = ctx.enter_context(tc.tile_pool(name="scP", bufs=2, space="PSUM"))
    accP = ctx.enter_context(tc.tile_pool(name="accP", bufs=2, space="PSUM"))
    miscP = ctx.enter_context(tc.tile_pool(name="miscP", bufs=2, space="PSUM"))
    expp = ctx.enter_context(tc.tile_pool(name="expp", bufs=6))
    smalls = ctx.enter_context(tc.tile_pool(name="smalls", bufs=2))
    outst = ctx.enter_context(tc.tile_pool(name="outst", bufs=3))

    # --- constants / weights / inputs ---
    ones_row = consts.tile([1, P], BF16)
    nc.vector.memset(ones_row, 1.0)

    w_sb = []
    xT = []
    for c in range(NDC):
        t = big.tile([P, 3 * DKC], BF16, name=f"w_sb{c}")
        nc.sync.dma_start(out=t, in_=wqkv_d[c * P:(c + 1) * P, :])
        w_sb.append(t)
        t = big.tile([P, S], BF16, name=f"xT{c}")
        nc.sync.dma_start(out=t, in_=xT_d[c * P:(c + 1) * P, :])
        xT.append(t)
    wproj_sb = []
    for t3 in range(3):
        t = big.tile([P, D], BF16, name=f"wproj_sb{t3}")
        nc.sync.dma_start(out=t, in_=wproj_d[t3 * P:(t3 + 1) * P, :])
        wproj_sb.append(t)

    bq_sb = consts.tile([P, 3], F32)
    nc.sync.dma_start(out=bq_sb, in_=bqkv_d[0:DKC].rearrange("(t p) -> p t", p=P))
    bk_sb = consts.tile([P, 3], F32)
    nc.sync.dma_start(out=bk_sb, in_=bqkv_d[DKC:2 * DKC].rearrange("(t p) -> p t", p=P))
    bv_row = consts.tile([1, DKC], F32)
    nc.sync.dma_start(out=bv_row, in_=bqkv_d[2 * DKC:3 * DKC].rearrange("(o f) -> o f", o=1))
    bv16 = consts.tile([1, DKC], BF16)
    nc.vector.tensor_copy(out=bv16, in_=bv_row)

    qT = [big.tile([P, S], BF16, name=f"qT{t}") for t in range(3)]
    kT = [big.tile([P, S], BF16, name=f"kT{t}") for t in range(3)]

    def emit_qk(which, t3, sb):
        # which: 0 = q, 1 = k.  Produces (q|k)T[t3][:, sb*512:(sb+1)*512].
        ps = miscP.tile([P, 512], F32, name="psqk", tag="miscP")
        for c in range(NDC):
            nc.tensor.matmul(
                ps,
                lhsT=(w_sb[c][:, which * DKC + t3 * P:which * DKC + (t3 + 1) * P]),
                rhs=(xT[c][:, sb * 512:(sb + 1) * 512]),
                start=(c == 0), stop=(c == NDC - 1))
        dst = (qT if which == 0 else kT)[t3][:, sb * 512:(sb + 1) * 512]
        bias = (bq_sb if which == 0 else bk_sb)[:, t3:t3 + 1]
        nc.vector.tensor_scalar(out=dst, in0=ps, scalar1=bias, scalar2=None, op0=ADD)

    def emit_v(st, vaug):
        ps = miscP.tile([P, DKC], F32, name="psv", tag="miscP")
        for c in range(NDC):
            nc.tensor.matmul(
                ps,
                lhsT=(xT[c][:, st * P:(st + 1) * P]),
                rhs=(w_sb[c][:, 2 * DKC:3 * DKC]),
                start=(c == 0), stop=False)
        nc.tensor.matmul(ps, lhsT=ones_row, rhs=bv16, start=False, stop=True)
        nc.vector.tensor_copy(
            out=vaug[:, :, st, 0:64],
            in_=ps.rearrange("p (h e) -> p h e", h=HPC))

    # --- qkv for pair 0 first so attention can start early; v and the
    # remaining pairs' k/q are produced inside the attention blocks ---
    vaug = big.tile([P, HPC, NST, P], BF16, name="vaug")
    nc.vector.memset(vaug[:, :, :, 64:128], 1.0)

    for sb in range(NSQ):
        emit_qk(1, 0, sb)        # kT[0]
    emit_qk(0, 0, 0)             # qT[0] @ sb0

    # --- attention + projection ---
    attnT = [big.tile([P, S], BF16, name=f"attnT{pr}") for pr in range(NPAIR)]

    def emit_av(pr, e, accA, accB, ck):
        nc.tensor.matmul(
            accA, lhsT=(vaug[:, 2 * pr, ck, :]), rhs=(e[:, 0:512]),
            start=(ck == 0), stop=(ck == NST - 1))
        nc.tensor.matmul(
            accB, lhsT=(vaug[:, 2 * pr + 1, ck, :]), rhs=(e[:, 512:1024]),
            start=(ck == 0), stop=(ck == NST - 1))

    def emit_attention_block(pr, sb, companion=None):
        accA = accP.tile([P, 512], F32, name="accA", tag="accP")
        accB = accP.tile([P, 512], F32, name="accB", tag="accP")
        prev = None
        for ck in range(NST):
            if companion is not None:
                companion(ck)
            sc = scP.tile([P, 1024], F32, name="sc", tag="scP")
            nc.tensor.matmul(
                sc[:, 0:512], lhsT=(kT[pr][0:64, ck * P:(ck + 1) * P]),
                rhs=(qT[pr][0:64, sb * 512:(sb + 1) * 512]),
                start=True, stop=True, tile_position=(0, 0))
            nc.tensor.matmul(
                sc[:, 512:1024], lhsT=(kT[pr][64:128, ck * P:(ck + 1) * P]),
                rhs=(qT[pr][64:128, sb * 512:(sb + 1) * 512]),
                start=True, stop=True, tile_position=(64, 0))
            if prev is not None:
                emit_av(pr, prev[1], accA, accB, prev[0])
            e = expp.tile([P, 1024], BF16, name="e", tag="expp")
            nc.scalar.activation(out=e, in_=sc, func=EXP)
            prev = (ck, e)
        emit_av(pr, prev[1], accA, accB, prev[0])

        # normalize into attnT; fast approx reciprocal (18 bits, plenty for
        # a softmax denominator) keeps the acc-bank release chain short
        for hh, acc in ((0, accA), (1, accB)):
            rec = smalls.tile([64, 512], F32, name="rec", tag="rec")
            nc.vector.reciprocal(out=rec, in_=acc[64:128, :])
            nc.vector.tensor_tensor(
                out=attnT[pr][64 * hh:64 * (hh + 1), sb * 512:(sb + 1) * 512],
                in0=acc[0:64, :], in1=rec, op=MULT)

    ostg_tiles = {}

    def emit_proj_piece(sb, idx):
        # idx in 0..7: piece (st, half); DMA the staged tile after half 1
        st = sb * 4 + idx // 2
        half = idx % 2
        if half == 0:
            ostg_tiles[st] = outst.tile([P, D], F32, name="ostg", tag="ostg")
        ostg = ostg_tiles[st]
        pp = miscP.tile([P, 384], F32, name="pp", tag="miscP")
        for t3 in range(3):
            nc.tensor.matmul(
                pp,
                lhsT=(attnT[t3][:, st * P:(st + 1) * P]),
                rhs=(wproj_sb[t3][:, half * 384:(half + 1) * 384]),
                start=(t3 == 0), stop=(t3 == 2))
        nc.vector.tensor_copy(out=ostg[:, half * 384:(half + 1) * 384], in_=pp)
        if half == 1:
            nc.sync.dma_start(out=out_d[st * P:(st + 1) * P, :], in_=ostg)

    def make_companion(pr, sb):
        # per-ck PE side-work emitted under this block's exp stream:
        #  - sb0: v chunks (block 0) and the next pair's k/q (blocks 0,1)
        #  - q for the next sb (each pair produces its own)
        #  - proj pieces of the previous sb, spread across the three blocks
        def companion(ck):
            if sb == 0:
                if pr == 0:
                    emit_v(ck, vaug)
                if pr < 2:
                    if ck in (1, 5, 9, 13):
                        emit_qk(1, pr + 1, (ck - 1) // 4)
                    elif ck == 14:
                        emit_qk(0, pr + 1, 0)
            if ck == 10 and sb + 1 < NSQ:
                emit_qk(0, pr, sb + 1)
            if sb > 0:
                base = pr * 3
                for j, ckp in enumerate((3, 7, 12)):
                    if ck == ckp and base + j < 8:
                        emit_proj_piece(sb - 1, base + j)
        return companion

    for sb in range(NSQ):
        for pr in range(NPAIR):
            emit_attention_block(pr, sb, make_companion(pr, sb))
    for idx in range(8):
        emit_proj_piece(NSQ - 1, idx)


_NC_CACHE = None


def _get_nc():
    global _NC_CACHE
    if _NC_CACHE is None:
        _NC_CACHE = build_kernel()
    return _NC_CACHE


def make_in_maps(hidden_states, W_attn, b_attn, W_proj, b_proj):
    in_maps = []
    scale = 1.0 / np.sqrt(HD)
    for c in range(8):
        b, g = c // 2, c % 2
        cols = slice(g * DKC, (g + 1) * DKC)
        wq = W_attn[:, 0 * D:1 * D][:, cols] * scale
        wk = W_attn[:, 1 * D:2 * D][:, cols]
        wv = W_attn[:, 2 * D:3 * D][:, cols]
        bq = b_attn[0 * D:1 * D][cols] * scale
        bk = b_attn[1 * D:2 * D][cols]
        bv = b_attn[2 * D:3 * D][cols]
        in_maps.append({
            "xT": np.ascontiguousarray(hidden_states[b].T).astype(ml_dtypes.bfloat16),
            "wqkv": np.ascontiguousarray(
                np.concatenate([wq, wk, wv], axis=1)).astype(ml_dtypes.bfloat16),
            "bqkv": np.ascontiguousarray(
                np.concatenate([bq, bk, bv]), dtype=np.float32),
            "wproj": np.ascontiguousarray(
                W_proj[g * DKC:(g + 1) * DKC, :]).astype(ml_dtypes.bfloat16),
        })
    return in_maps


def run(hidden_states, W_attn, b_attn, W_proj, b_proj, trace=False):
    nc = _get_nc()
    in_maps = make_in_maps(hidden_states, W_attn, b_attn, W_proj, b_proj)
    res = run_bass_kernel_spmd(nc, in_maps, core_ids=list(range(8)), trace=trace)
    out = np.empty((B, S, D), dtype=np.float32)
    for b in range(B):
        out[b] = res.results[2 * b]["out"] + res.results[2 * b + 1]["out"] + b_proj
    return out, res


def kernel(hidden_states, W_attn, b_attn, W_proj, b_proj):
    hidden_states = np.asarray(hidden_states, dtype=np.float32)
    W_attn = np.asarray(W_attn, dtype=np.float32)
    b_attn = np.asarray(b_attn, dtype=np.float32)
    W_proj = np.asarray(W_proj, dtype=np.float32)
    b_proj = np.asarray(b_proj, dtype=np.float32)
    out, _ = run(hidden_states, W_attn, b_attn, W_proj, b_proj, trace=False)
    return out


# revision 12
# speedup vs baseline: 1.1542x; 1.1542x over previous
"""GPT2 attention (B=4, S=2048, D=768, H=12, no causal mask) on 8 trn2 cores.

Sharding: core c -> batch b = c//2, head-group g = c%2 (6 heads of 64).
Each core computes its 6 heads' attention + the matching row-block of the
output projection; host sums the two per-batch partials and adds b_proj.

Host-side prep: x is pre-transposed to xT [768, 2048] (so no PE transposes
on device) and the 1/sqrt(hd) scale is folded into W_q / b_q.

On-chip layout (per core):
  xT    [768, 2048] bf16 (6 tiles [128, S]) -- loaded directly
  qT/kT [384, 2048] (3 tiles [128, S], one head pair per tile)
  vaug  [128, 6, 16, 128] -- v natural [sk, hd] per (head, sk-chunk),
        cols 64:128 = ones (denominator rows of the AV accumulation)
  scores per (pair, sq-block, sk-chunk): one psum tile [128 sk, 1024]
        (head A in cols 0:512, head B in 512:1024; 2 banks), exp'd by a
        single ACT instruction into SBUF bf16, then two AV accumulation
        matmuls with vaug (rows 64:127 of acc = softmax denominator).
  Normalize: DVE reciprocal + DVE multiply into attnT (natural lhsT for
  the projection).  proj: out [S, 768] partial to DRAM per sq tile.

The attention phase is ACT(exp)-bound (~1.1us per [128,1024] exp); qkv/proj
matmuls are emitted after the attention steps of each block so the
scheduler slots them into PE idle time under the exp stream.
"""

import json
from contextlib import ExitStack

import ml_dtypes
import numpy as np

import concourse.bass as bass
import concourse.mybir as mybir
import concourse.tile as tile
from concourse import library_config
from concourse.bass_utils import run_bass_kernel_spmd

B, S, D = 4, 2048, 768
H, HD = 12, 64
HPC = 6            # heads per core
DKC = HPC * HD     # 384: per-core width of q/k/v
NPAIR = HPC // 2   # 3 head pairs
P = 128
F32 = mybir.dt.float32
BF16 = mybir.dt.bfloat16

NSQ = S // 512     # 4 sq blocks
NST = S // 128     # 16 s tiles
NDC = D // 128     # 6 d chunks


def _split_multi_waits(bir_bytes):
    """Walrus in this toolchain accepts only one sync-wait per instruction.

    Hoist extra waits onto same-engine NoOps inserted just before. Engines
    execute their stream in order and semaphores are monotonic, so this is
    semantically identical.
    """
    m = json.loads(bir_bytes)
    n_split = 0
    for fn in m["functions"]:
        for blk in fn["blocks"]:
            new = []
            for ins in blk["instructions"]:
                si = ins.get("sync_info")
                waits = (si or {}).get("on_wait") or []
                if len(waits) > 1:
                    n_split += 1
                    for j, w in enumerate(waits[:-1]):
                        new.append({
                            "debug": ins.get("debug", 0),
                            "engine": ins["engine"],
                            "ins": [], "outs": [],
                            "name": f"{ins['name']}w{j}",
                            "opcode": "NoOp",
                            "sync_info": {"on_update": [], "on_wait": [w]},
                        })
                    si["on_wait"] = [waits[-1]]
                new.append(ins)
            blk["instructions"] = new
    return json.dumps(m).encode()


def build_kernel():
    nc = bass.Bass("TRN2", target_bir_lowering=False, debug=False)
    xT_d = nc.dram_tensor("xT", [D, S], BF16, kind="ExternalInput").ap()
    wqkv_d = nc.dram_tensor("wqkv", [D, 3 * DKC], BF16, kind="ExternalInput").ap()
    bqkv_d = nc.dram_tensor("bqkv", [3 * DKC], F32, kind="ExternalInput").ap()
    wproj_d = nc.dram_tensor("wproj", [DKC, D], BF16, kind="ExternalInput").ap()
    out_d = nc.dram_tensor("out", [S, D], F32, kind="ExternalOutput").ap()

    with tile.TileContext(nc) as tc:
        with ExitStack() as ctx:
            _body(ctx, tc, xT_d, wqkv_d, bqkv_d, wproj_d, out_d)
    orig_to_json = nc.to_json_bytes
    nc.to_json_bytes = lambda: _split_multi_waits(orig_to_json())
    return nc


def _body(ctx, tc, xT_d, wqkv_d, bqkv_d, wproj_d, out_d):
    nc = tc.nc
    ADD = mybir.AluOpType.add
    MULT = mybir.AluOpType.mult
    EXP = mybir.ActivationFunctionType.Exp

    consts = ctx.enter_context(tc.tile_pool(name="consts", bufs=1))
    big = ctx.enter_context(tc.tile_pool(name="big", bufs=1))
    # PSUM: exactly 8 banks: scores 2x[128,1024] + acc 2x[128,512] + misc 2
    scP = ctx.enter_context(tc.tile_pool(name="scP", bufs=2, space="PSUM"))
    accP = ctx.enter_context(tc.tile_pool(name="accP", bufs=2, space="PSUM"))
    miscP = ctx.enter_context(tc.tile_pool(name="miscP", bufs=2, space="PSUM"))
    expp = ctx.enter_context(tc.tile_pool(name="expp", bufs=8))
    smalls = ctx.enter_context(tc.tile_pool(name="smalls", bufs=2))
    outst = ctx.enter_context(tc.tile_pool(name="outst", bufs=3))

    # --- constants / weights / inputs ---
    ones_row = consts.tile([1, P], BF16)
    nc.vector.memset(ones_row, 1.0)

    w_sb = []
    xT = []
    for c in range(NDC):
        t = big.tile([P, 3 * DKC], BF16, name=f"w_sb{c}")
        nc.sync.dma_start(out=t, in_=wqkv_d[c * P:(c + 1) * P, :])
        w_sb.append(t)
        t = big.tile([P, S], BF16, name=f"xT{c}")
        nc.sync.dma_start(out=t, in_=xT_d[c * P:(c + 1) * P, :])
        xT.append(t)
    wproj_sb = []
    for t3 in range(3):
        t = big.tile([P, D], BF16, name=f"wproj_sb{t3}")
        nc.sync.dma_start(out=t, in_=wproj_d[t3 * P:(t3 + 1) * P, :])
        wproj_sb.append(t)

    bq_sb = consts.tile([P, 3], F32)
    nc.sync.dma_start(out=bq_sb, in_=bqkv_d[0:DKC].rearrange("(t p) -> p t", p=P))
    bk_sb = consts.tile([P, 3], F32)
    nc.sync.dma_start(out=bk_sb, in_=bqkv_d[DKC:2 * DKC].rearrange("(t p) -> p t", p=P))
    bv_row = consts.tile([1, DKC], F32)
    nc.sync.dma_start(out=bv_row, in_=bqkv_d[2 * DKC:3 * DKC].rearrange("(o f) -> o f", o=1))
    bv16 = consts.tile([1, DKC], BF16)
    nc.vector.tensor_copy(out=bv16, in_=bv_row)

    qT = [big.tile([P, S], BF16, name=f"qT{t}") for t in range(3)]
    kT = [big.tile([P, S], BF16, name=f"kT{t}") for t in range(3)]

    def emit_qk(which, t3, sb):
        # which: 0 = q, 1 = k.  Produces (q|k)T[t3][:, sb*512:(sb+1)*512].
        ps = miscP.tile([P, 512], F32, name="psqk", tag="miscP")
        for c in range(NDC):
            nc.tensor.matmul(
                ps,
                lhsT=(w_sb[c][:, which * DKC + t3 * P:which * DKC + (t3 + 1) * P]),
                rhs=(xT[c][:, sb * 512:(sb + 1) * 512]),
                start=(c == 0), stop=(c == NDC - 1))
        dst = (qT if which == 0 else kT)[t3][:, sb * 512:(sb + 1) * 512]
        bias = (bq_sb if which == 0 else bk_sb)[:, t3:t3 + 1]
        nc.vector.tensor_scalar(out=dst, in0=ps, scalar1=bias, scalar2=None, op0=ADD)

    def emit_v(st, vaug):
        ps = miscP.tile([P, DKC], F32, name="psv", tag="miscP")
        for c in range(NDC):
            nc.tensor.matmul(
                ps,
                lhsT=(xT[c][:, st * P:(st + 1) * P]),
                rhs=(w_sb[c][:, 2 * DKC:3 * DKC]),
                start=(c == 0), stop=False)
        nc.tensor.matmul(ps, lhsT=ones_row, rhs=bv16, start=False, stop=True)
        nc.vector.tensor_copy(
            out=vaug[:, :, st, 0:64],
            in_=ps.rearrange("p (h e) -> p h e", h=HPC))

    # --- qkv for pair 0 first so attention can start early; v and the
    # remaining pairs' k/q are produced inside the attention blocks ---
    vaug = big.tile([P, HPC, NST, P], BF16, name="vaug")
    nc.vector.memset(vaug[:, :, :, 64:128], 1.0)

    for sb in range(NSQ):
        emit_qk(1, 0, sb)        # kT[0]
    emit_qk(0, 0, 0)             # qT[0] @ sb0

    # --- attention + projection ---
    attnT = [big.tile([P, S], BF16, name=f"attnT{pr}") for pr in range(NPAIR)]

    def emit_av(pr, e, accA, accB, ck):
        nc.tensor.matmul(
            accA, lhsT=(vaug[:, 2 * pr, ck, :]), rhs=(e[:, 0:512]),
            start=(ck == 0), stop=(ck == NST - 1))
        nc.tensor.matmul(
            accB, lhsT=(vaug[:, 2 * pr + 1, ck, :]), rhs=(e[:, 512:1024]),
            start=(ck == 0), stop=(ck == NST - 1))

    def emit_attention_block(pr, sb, companion=None):
        accA = accP.tile([P, 512], F32, name="accA", tag="accP")
        accB = accP.tile([P, 512], F32, name="accB", tag="accP")
        prev = None
        for ck in range(NST):
            if companion is not None:
                companion(ck)
            sc = scP.tile([P, 1024], F32, name="sc", tag="scP")
            nc.tensor.matmul(
                sc[:, 0:512], lhsT=(kT[pr][0:64, ck * P:(ck + 1) * P]),
                rhs=(qT[pr][0:64, sb * 512:(sb + 1) * 512]),
                start=True, stop=True, tile_position=(0, 0))
            nc.tensor.matmul(
                sc[:, 512:1024], lhsT=(kT[pr][64:128, ck * P:(ck + 1) * P]),
                rhs=(qT[pr][64:128, sb * 512:(sb + 1) * 512]),
                start=True, stop=True, tile_position=(64, 0))
            if prev is not None:
                emit_av(pr, prev[1], accA, accB, prev[0])
            e = expp.tile([P, 1024], BF16, name="e", tag="expp")
            nc.scalar.activation(out=e, in_=sc, func=EXP)
            prev = (ck, e)
        emit_av(pr, prev[1], accA, accB, prev[0])

        # Drain acc psum to SBUF with cheap copies (releases the acc banks
        # fast so the next block's AV matmuls don't stall the PE FIFO); the
        # expensive reciprocal runs off the critical path, partition-packed
        # so one op covers both heads.
        vals = smalls.tile([P, 512], F32, name="vals", tag="vals")
        den = smalls.tile([P, 512], F32, name="den", tag="den")
        nc.vector.tensor_copy(out=vals[0:64, :], in_=accA[0:64, :])
        nc.vector.tensor_copy(out=den[0:64, :], in_=accA[64:128, :])
        nc.vector.tensor_copy(out=vals[64:128, :], in_=accB[0:64, :])
        nc.vector.tensor_copy(out=den[64:128, :], in_=accB[64:128, :])
        rec = smalls.tile([P, 512], F32, name="rec", tag="rec")
        nc.vector.reciprocal(out=rec, in_=den)
        nc.vector.tensor_tensor(
            out=attnT[pr][:, sb * 512:(sb + 1) * 512],
            in0=vals, in1=rec, op=MULT)

    ostg_tiles = {}

    def emit_proj_piece(sb, idx):
        # idx in 0..7: piece (st, half); DMA the staged tile after half 1
        st = sb * 4 + idx // 2
        half = idx % 2
        if half == 0:
            ostg_tiles[st] = outst.tile([P, D], F32, name="ostg", tag="ostg")
        ostg = ostg_tiles[st]
        pp = miscP.tile([P, 384], F32, name="pp", tag="miscP")
        for t3 in range(3):
            nc.tensor.matmul(
                pp,
                lhsT=(attnT[t3][:, st * P:(st + 1) * P]),
                rhs=(wproj_sb[t3][:, half * 384:(half + 1) * 384]),
                start=(t3 == 0), stop=(t3 == 2))
        nc.vector.tensor_copy(out=ostg[:, half * 384:(half + 1) * 384], in_=pp)
        if half == 1:
            nc.sync.dma_start(out=out_d[st * P:(st + 1) * P, :], in_=ostg)

    def make_companion(pr, sb):
        # per-ck PE side-work emitted under this block's exp stream:
        #  - sb0: v chunks (block 0) and the next pair's k/q (blocks 0,1)
        #  - q for the next sb (each pair produces its own)
        #  - proj pieces of the previous sb, spread across the three blocks
        def companion(ck):
            if sb == 0:
                if pr == 0:
                    emit_v(ck, vaug)
                if pr < 2:
                    if ck in (1, 5, 9, 13):
                        emit_qk(1, pr + 1, (ck - 1) // 4)
                    elif ck == 14:
                        emit_qk(0, pr + 1, 0)
            if ck == 10 and sb + 1 < NSQ:
                emit_qk(0, pr, sb + 1)
            if sb > 0:
                base = pr * 3
                for j, ckp in enumerate((3, 7, 12)):
                    if ck == ckp and base + j < 8:
                        emit_proj_piece(sb - 1, base + j)
        return companion

    for sb in range(NSQ):
        for pr in range(NPAIR):
            emit_attention_block(pr, sb, make_companion(pr, sb))
    for idx in range(8):
        emit_proj_piece(NSQ - 1, idx)


_NC_CACHE = None


def _get_nc():
    global _NC_CACHE
    if _NC_CACHE is None:
        _NC_CACHE = build_kernel()
    return _NC_CACHE


def make_in_maps(hidden_states, W_attn, b_attn, W_proj, b_proj):
    in_maps = []
    scale = 1.0 / np.sqrt(HD)
    for c in range(8):
        b, g = c // 2, c % 2
        cols = slice(g * DKC, (g + 1) * DKC)
        wq = W_attn[:, 0 * D:1 * D][:, cols] * scale
        wk = W_attn[:, 1 * D:2 * D][:, cols]
        wv = W_attn[:, 2 * D:3 * D][:, cols]
        bq = b_attn[0 * D:1 * D][cols] * scale
        bk = b_attn[1 * D:2 * D][cols]
        bv = b_attn[2 * D:3 * D][cols]
        in_maps.append({
            "xT": np.ascontiguousarray(hidden_states[b].T).astype(ml_dtypes.bfloat16),
            "wqkv": np.ascontiguousarray(
                np.concatenate([wq, wk, wv], axis=1)).astype(ml_dtypes.bfloat16),
            "bqkv": np.ascontiguousarray(
                np.concatenate([bq, bk, bv]), dtype=np.float32),
            "wproj": np.ascontiguousarray(
                W_proj[g * DKC:(g + 1) * DKC, :]).astype(ml_dtypes.bfloat16),
        })
    return in_maps


def run(hidden_states, W_attn, b_attn, W_proj, b_proj, trace=False):
    nc = _get_nc()
    in_maps = make_in_maps(hidden_states, W_attn, b_attn, W_proj, b_proj)
    res = run_bass_kernel_spmd(nc, in_maps, core_ids=list(range(8)), trace=trace)
    out = np.empty((B, S, D), dtype=np.float32)
    for b in range(B):
        out[b] = res.results[2 * b]["out"] + res.results[2 * b + 1]["out"] + b_proj
    return out, res


def kernel(hidden_states, W_attn, b_attn, W_proj, b_proj):
    hidden_states = np.asarray(hidden_states, dtype=np.float32)
    W_attn = np.asarray(W_attn, dtype=np.float32)
    b_attn = np.asarray(b_attn, dtype=np.float32)
    W_proj = np.asarray(W_proj, dtype=np.float32)
    b_proj = np.asarray(b_proj, dtype=np.float32)
    out, _ = run(hidden_states, W_attn, b_attn, W_proj, b_proj, trace=False)
    return out


# revision 17
# speedup vs baseline: 1.2515x; 1.0843x over previous
"""GPT2 attention (B=4, S=2048, D=768, H=12, no causal mask) on 8 trn2 cores.

Sharding: core c -> batch b = c//2, head-group g = c%2 (6 heads of 64).
Each core computes its 6 heads' attention + the matching row-block of the
output projection; host sums the two per-batch partials and adds b_proj.

Host-side prep: x is pre-transposed to xT [768, 2048] (so no PE transposes
on device) and the 1/sqrt(hd) scale is folded into W_q / b_q.

On-chip layout (per core):
  xT    [768, 2048] bf16 (6 tiles [128, S]) -- loaded directly
  qT/kT [384, 2048] (3 tiles [128, S], one head pair per tile)
  vaug  [128, 6, 16, 128] -- v natural [sk, hd] per (head, sk-chunk),
        cols 64:128 = ones (denominator rows of the AV accumulation)
  scores per (pair, sq-block, sk-chunk): one psum tile [128 sk, 1024]
        (head A in cols 0:512, head B in 512:1024; 2 banks), exp'd by a
        single ACT instruction into SBUF bf16, then two AV accumulation
        matmuls with vaug (rows 64:127 of acc = softmax denominator).
  Normalize: DVE reciprocal + DVE multiply into attnT (natural lhsT for
  the projection).  proj: out [S, 768] partial to DRAM per sq tile.

The attention phase is ACT(exp)-bound (~1.1us per [128,1024] exp); qkv/proj
matmuls are emitted after the attention steps of each block so the
scheduler slots them into PE idle time under the exp stream.
"""

import json
from contextlib import ExitStack

import ml_dtypes
import numpy as np

import concourse.bass as bass
import concourse.mybir as mybir
import concourse.tile as tile
from concourse import library_config
from concourse.bass_utils import run_bass_kernel_spmd

B, S, D = 4, 2048, 768
H, HD = 12, 64
HPC = 6            # heads per core
DKC = HPC * HD     # 384: per-core width of q/k/v
NPAIR = HPC // 2   # 3 head pairs
P = 128
F32 = mybir.dt.float32
BF16 = mybir.dt.bfloat16

NSQ = S // 512     # 4 sq blocks
NST = S // 128     # 16 s tiles
NDC = D // 128     # 6 d chunks


def _split_multi_waits(bir_bytes):
    """Walrus in this toolchain accepts only one sync-wait per instruction.

    Hoist extra waits onto same-engine NoOps inserted just before. Engines
    execute their stream in order and semaphores are monotonic, so this is
    semantically identical.
    """
    m = json.loads(bir_bytes)
    n_split = 0
    for fn in m["functions"]:
        for blk in fn["blocks"]:
            new = []
            for ins in blk["instructions"]:
                si = ins.get("sync_info")
                waits = (si or {}).get("on_wait") or []
                if len(waits) > 1:
                    n_split += 1
                    for j, w in enumerate(waits[:-1]):
                        new.append({
                            "debug": ins.get("debug", 0),
                            "engine": ins["engine"],
                            "ins": [], "outs": [],
                            "name": f"{ins['name']}w{j}",
                            "opcode": "NoOp",
                            "sync_info": {"on_update": [], "on_wait": [w]},
                        })
                    si["on_wait"] = [waits[-1]]
                new.append(ins)
            blk["instructions"] = new
    return json.dumps(m).encode()


def build_kernel():
    nc = bass.Bass("TRN2", target_bir_lowering=False, debug=False)
    xT_d = nc.dram_tensor("xT", [D, S], BF16, kind="ExternalInput").ap()
    wqkv_d = nc.dram_tensor("wqkv", [D, 3 * DKC], BF16, kind="ExternalInput").ap()
    bqkv_d = nc.dram_tensor("bqkv", [3 * DKC], F32, kind="ExternalInput").ap()
    wproj_d = nc.dram_tensor("wproj", [DKC, D], BF16, kind="ExternalInput").ap()
    out_d = nc.dram_tensor("out", [S, D], F32, kind="ExternalOutput").ap()

    with tile.TileContext(nc) as tc:
        with ExitStack() as ctx:
            _body(ctx, tc, xT_d, wqkv_d, bqkv_d, wproj_d, out_d)
    orig_to_json = nc.to_json_bytes
    nc.to_json_bytes = lambda: _split_multi_waits(orig_to_json())
    return nc


def _body(ctx, tc, xT_d, wqkv_d, bqkv_d, wproj_d, out_d):
    nc = tc.nc
    ADD = mybir.AluOpType.add
    MULT = mybir.AluOpType.mult
    EXP = mybir.ActivationFunctionType.Exp

    consts = ctx.enter_context(tc.tile_pool(name="consts", bufs=1))
    big = ctx.enter_context(tc.tile_pool(name="big", bufs=1))
    # PSUM: exactly 8 banks: scores 2x[128,1024] + acc 2x[128,512] + misc 2
    scP = ctx.enter_context(tc.tile_pool(name="scP", bufs=2, space="PSUM"))
    accP = ctx.enter_context(tc.tile_pool(name="accP", bufs=2, space="PSUM"))
    miscP = ctx.enter_context(tc.tile_pool(name="miscP", bufs=2, space="PSUM"))
    expp = ctx.enter_context(tc.tile_pool(name="expp", bufs=8))
    smalls = ctx.enter_context(tc.tile_pool(name="smalls", bufs=2))
    outst = ctx.enter_context(tc.tile_pool(name="outst", bufs=3))

    # --- constants / weights / inputs ---
    ones_row = consts.tile([1, P], BF16)
    nc.vector.memset(ones_row, 1.0)

    # Dummy matmuls: keep the PE busy during the initial DMA so the HAM
    # clock-gate is warm (2.4 GHz) when the real qkv matmuls start.
    for _ in range(120):
        warm = miscP.tile([P, P], F32, name="warm", tag="miscP")
        nc.tensor.matmul(warm, lhsT=ones_row, rhs=ones_row, start=True, stop=True)

    w_sb = []
    xT = []
    for c in range(NDC):
        t = big.tile([P, 3 * DKC], BF16, name=f"w_sb{c}")
        nc.sync.dma_start(out=t, in_=wqkv_d[c * P:(c + 1) * P, :])
        w_sb.append(t)
        t = big.tile([P, S], BF16, name=f"xT{c}")
        nc.sync.dma_start(out=t, in_=xT_d[c * P:(c + 1) * P, :])
        xT.append(t)
    wproj_sb = []
    for t3 in range(3):
        t = big.tile([P, D], BF16, name=f"wproj_sb{t3}")
        nc.sync.dma_start(out=t, in_=wproj_d[t3 * P:(t3 + 1) * P, :])
        wproj_sb.append(t)

    bq_sb = consts.tile([P, 3], F32)
    nc.sync.dma_start(out=bq_sb, in_=bqkv_d[0:DKC].rearrange("(t p) -> p t", p=P))
    bk_sb = consts.tile([P, 3], F32)
    nc.sync.dma_start(out=bk_sb, in_=bqkv_d[DKC:2 * DKC].rearrange("(t p) -> p t", p=P))
    # v-bias is NOT applied on device: softmax weights sum to 1, so
    # attn(v + b_v) = attn(v) + b_v, and b_v @ W_proj is added on the host.

    qT = [big.tile([P, S], BF16, name=f"qT{t}") for t in range(3)]
    kT = [big.tile([P, S], BF16, name=f"kT{t}") for t in range(3)]

    def emit_qk(which, t3, sb):
        # which: 0 = q, 1 = k.  Produces (q|k)T[t3][:, sb*512:(sb+1)*512].
        ps = miscP.tile([P, 512], F32, name="psqk", tag="miscP")
        for c in range(NDC):
            nc.tensor.matmul(
                ps,
                lhsT=(w_sb[c][:, which * DKC + t3 * P:which * DKC + (t3 + 1) * P]),
                rhs=(xT[c][:, sb * 512:(sb + 1) * 512]),
                start=(c == 0), stop=(c == NDC - 1))
        dst = (qT if which == 0 else kT)[t3][:, sb * 512:(sb + 1) * 512]
        bias = (bq_sb if which == 0 else bk_sb)[:, t3:t3 + 1]
        nc.vector.tensor_scalar(out=dst, in0=ps, scalar1=bias, scalar2=None, op0=ADD)

    def emit_v(st, vaug):
        ps = miscP.tile([P, DKC], F32, name="psv", tag="miscP")
        for c in range(NDC):
            nc.tensor.matmul(
                ps,
                lhsT=(xT[c][:, st * P:(st + 1) * P]),
                rhs=(w_sb[c][:, 2 * DKC:3 * DKC]),
                start=(c == 0), stop=(c == NDC - 1))
        nc.vector.tensor_copy(
            out=vaug[:, :, st, 0:64],
            in_=ps.rearrange("p (h e) -> p h e", h=HPC))

    # --- qkv for pair 0 first so attention can start early; v and the
    # remaining pairs' k/q are produced inside the attention blocks ---
    vaug = big.tile([P, HPC, NST, P], BF16, name="vaug")
    nc.vector.memset(vaug[:, :, :, 64:128], 1.0)

    for sb in range(NSQ):
        emit_qk(1, 0, sb)        # kT[0]
    emit_qk(0, 0, 0)             # qT[0] @ sb0

    # --- attention + projection ---
    attnT = [big.tile([P, S], BF16, name=f"attnT{pr}") for pr in range(NPAIR)]

    def emit_av(pr, e, accA, accB, ck):
        nc.tensor.matmul(
            accA, lhsT=(vaug[:, 2 * pr, ck, :]), rhs=(e[:, 0:512]),
            start=(ck == 0), stop=(ck == NST - 1))
        nc.tensor.matmul(
            accB, lhsT=(vaug[:, 2 * pr + 1, ck, :]), rhs=(e[:, 512:1024]),
            start=(ck == 0), stop=(ck == NST - 1))

    AV_LAG = 3   # AV(ck) emitted after scores(ck+AV_LAG): the first AVs of a
    # block (which wait on the previous block's acc-bank release) must not
    # block the scores matmuls feeding the exp stream in the PE FIFO.

    def emit_attention_block(pr, sb, companion=None):
        accA = accP.tile([P, 512], F32, name="accA", tag="accP")
        accB = accP.tile([P, 512], F32, name="accB", tag="accP")
        pend = []
        for ck in range(NST):
            if companion is not None:
                companion(ck)
            sc = scP.tile([P, 1024], F32, name="sc", tag="scP")
            nc.tensor.matmul(
                sc[:, 0:512], lhsT=(kT[pr][0:64, ck * P:(ck + 1) * P]),
                rhs=(qT[pr][0:64, sb * 512:(sb + 1) * 512]),
                start=True, stop=True, tile_position=(0, 0))
            nc.tensor.matmul(
                sc[:, 512:1024], lhsT=(kT[pr][64:128, ck * P:(ck + 1) * P]),
                rhs=(qT[pr][64:128, sb * 512:(sb + 1) * 512]),
                start=True, stop=True, tile_position=(64, 0))
            if len(pend) >= AV_LAG:
                cp, ep = pend.pop(0)
                emit_av(pr, ep, accA, accB, cp)
            e = expp.tile([P, 1024], BF16, name="e", tag="expp")
            nc.scalar.activation(out=e, in_=sc, func=EXP)
            pend.append((ck, e))
        for cp, ep in pend:
            emit_av(pr, ep, accA, accB, cp)

        # Drain acc psum to SBUF with cheap copies (releases the acc banks
        # fast so the next block's AV matmuls don't stall the PE FIFO); the
        # expensive reciprocal runs off the critical path, partition-packed
        # so one op covers both heads.
        vals = smalls.tile([P, 512], F32, name="vals", tag="vals")
        den = smalls.tile([P, 512], F32, name="den", tag="den")
        nc.vector.tensor_copy(out=vals[0:64, :], in_=accA[0:64, :])
        nc.vector.tensor_copy(out=den[0:64, :], in_=accA[64:128, :])
        nc.vector.tensor_copy(out=vals[64:128, :], in_=accB[0:64, :])
        nc.vector.tensor_copy(out=den[64:128, :], in_=accB[64:128, :])
        rec = smalls.tile([P, 512], F32, name="rec", tag="rec")
        nc.vector.reciprocal(out=rec, in_=den)
        nc.vector.tensor_tensor(
            out=attnT[pr][:, sb * 512:(sb + 1) * 512],
            in0=vals, in1=rec, op=MULT)

    ostg_tiles = {}

    def emit_proj_piece(sb, idx):
        # idx in 0..7: piece (st, half); DMA the staged tile after half 1
        st = sb * 4 + idx // 2
        half = idx % 2
        if half == 0:
            ostg_tiles[st] = outst.tile([P, D], F32, name="ostg", tag="ostg")
        ostg = ostg_tiles[st]
        pp = miscP.tile([P, 384], F32, name="pp", tag="miscP")
        for t3 in range(3):
            nc.tensor.matmul(
                pp,
                lhsT=(attnT[t3][:, st * P:(st + 1) * P]),
                rhs=(wproj_sb[t3][:, half * 384:(half + 1) * 384]),
                start=(t3 == 0), stop=(t3 == 2))
        nc.vector.tensor_copy(out=ostg[:, half * 384:(half + 1) * 384], in_=pp)
        if half == 1:
            nc.sync.dma_start(out=out_d[st * P:(st + 1) * P, :], in_=ostg)

    def make_companion(pr, sb):
        # per-ck PE side-work emitted under this block's exp stream:
        #  - sb0: v chunks (block 0) and the next pair's k/q (blocks 0,1)
        #  - q for the next sb (each pair produces its own)
        #  - proj pieces of the previous sb, spread across the three blocks
        def companion(ck):
            if sb == 0:
                if pr == 0:
                    emit_v(ck, vaug)
                if pr < 2:
                    if ck in (1, 5, 9, 13):
                        emit_qk(1, pr + 1, (ck - 1) // 4)
                    elif ck == 14:
                        emit_qk(0, pr + 1, 0)
            if ck == 10 and sb + 1 < NSQ:
                emit_qk(0, pr, sb + 1)
            if sb > 0:
                base = pr * 3
                for j, ckp in enumerate((3, 7, 12)):
                    if ck == ckp and base + j < 8:
                        emit_proj_piece(sb - 1, base + j)
        return companion

    for sb in range(NSQ):
        for pr in range(NPAIR):
            emit_attention_block(pr, sb, make_companion(pr, sb))
    for idx in range(8):
        emit_proj_piece(NSQ - 1, idx)


_NC_CACHE = None


def _get_nc():
    global _NC_CACHE
    if _NC_CACHE is None:
        _NC_CACHE = build_kernel()
    return _NC_CACHE


def make_in_maps(hidden_states, W_attn, b_attn, W_proj, b_proj):
    in_maps = []
    scale = 1.0 / np.sqrt(HD)
    for c in range(8):
        b, g = c // 2, c % 2
        cols = slice(g * DKC, (g + 1) * DKC)
        wq = W_attn[:, 0 * D:1 * D][:, cols] * scale
        wk = W_attn[:, 1 * D:2 * D][:, cols]
        wv = W_attn[:, 2 * D:3 * D][:, cols]
        bq = b_attn[0 * D:1 * D][cols] * scale
        bk = b_attn[1 * D:2 * D][cols]
        bv = b_attn[2 * D:3 * D][cols]
        in_maps.append({
            "xT": np.ascontiguousarray(hidden_states[b].T).astype(ml_dtypes.bfloat16),
            "wqkv": np.ascontiguousarray(
                np.concatenate([wq, wk, wv], axis=1)).astype(ml_dtypes.bfloat16),
            "bqkv": np.ascontiguousarray(
                np.concatenate([bq, bk, bv]), dtype=np.float32),
            "wproj": np.ascontiguousarray(
                W_proj[g * DKC:(g + 1) * DKC, :]).astype(ml_dtypes.bfloat16),
        })
    return in_maps


def run(hidden_states, W_attn, b_attn, W_proj, b_proj, trace=False):
    nc = _get_nc()
    in_maps = make_in_maps(hidden_states, W_attn, b_attn, W_proj, b_proj)
    res = run_bass_kernel_spmd(nc, in_maps, core_ids=list(range(8)), trace=trace)
    # v-bias: attn(v + b_v) = attn(v) + b_v since softmax weights sum to 1,
    # so its projection lands as a constant row added host-side.
    bv_proj = np.asarray(b_attn[2 * D:3 * D], dtype=np.float32) @ np.asarray(
        W_proj, dtype=np.float32)
    out = np.empty((B, S, D), dtype=np.float32)
    for b in range(B):
        out[b] = (res.results[2 * b]["out"] + res.results[2 * b + 1]["out"]
                  + b_proj + bv_proj)
    return out, res


def kernel(hidden_states, W_attn, b_attn, W_proj, b_proj):
    hidden_states = np.asarray(hidden_states, dtype=np.float32)
    W_attn = np.asarray(W_attn, dtype=np.float32)
    b_attn = np.asarray(b_attn, dtype=np.float32)
    W_proj = np.asarray(W_proj, dtype=np.float32)
    b_proj = np.asarray(b_proj, dtype=np.float32)
    out, _ = run(hidden_states, W_attn, b_attn, W_proj, b_proj, trace=False)
    return out


# revision 23
# speedup vs baseline: 1.2895x; 1.0304x over previous
"""GPT2 attention (B=4, S=2048, D=768, H=12, no causal mask) on 8 trn2 cores.

Sharding: core c -> batch b = c//2, head-group g = c%2 (6 heads of 64).
Each core computes its 6 heads' attention + the matching row-block of the
output projection; host sums the two per-batch partials and adds b_proj.

Host-side prep: x is pre-transposed to xT [768, 2048] (so no PE transposes
on device) and the 1/sqrt(hd) scale is folded into W_q / b_q.

On-chip layout (per core):
  xT    [768, 2048] bf16 (6 tiles [128, S]) -- loaded directly
  qT/kT [384, 2048] (3 tiles [128, S], one head pair per tile)
  vaug  [128, 6, 16, 128] -- v natural [sk, hd] per (head, sk-chunk),
        cols 64:128 = ones (denominator rows of the AV accumulation)
  scores per (pair, sq-block, sk-chunk): one psum tile [128 sk, 1024]
        (head A in cols 0:512, head B in 512:1024; 2 banks), exp'd by a
        single ACT instruction into SBUF bf16, then two AV accumulation
        matmuls with vaug (rows 64:127 of acc = softmax denominator).
  Normalize: DVE reciprocal + DVE multiply into attnT (natural lhsT for
  the projection).  proj: out [S, 768] partial to DRAM per sq tile.

The attention phase is ACT(exp)-bound (~1.1us per [128,1024] exp); qkv/proj
matmuls are emitted after the attention steps of each block so the
scheduler slots them into PE idle time under the exp stream.
"""

import json
from contextlib import ExitStack

import ml_dtypes
import numpy as np

import concourse.bass as bass
import concourse.mybir as mybir
import concourse.tile as tile
from concourse import library_config
from concourse.bass_utils import run_bass_kernel_spmd

B, S, D = 4, 2048, 768
H, HD = 12, 64
HPC = 6            # heads per core
DKC = HPC * HD     # 384: per-core width of q/k/v
NPAIR = HPC // 2   # 3 head pairs
P = 128
F32 = mybir.dt.float32
BF16 = mybir.dt.bfloat16

NSQ = S // 512     # 4 sq blocks
NST = S // 128     # 16 s tiles
NDC = D // 128     # 6 d chunks


def _split_multi_waits(bir_bytes):
    """Walrus in this toolchain accepts only one sync-wait per instruction.

    Hoist extra waits onto same-engine NoOps inserted just before. Engines
    execute their stream in order and semaphores are monotonic, so this is
    semantically identical.
    """
    m = json.loads(bir_bytes)
    n_split = 0
    for fn in m["functions"]:
        for blk in fn["blocks"]:
            new = []
            for ins in blk["instructions"]:
                si = ins.get("sync_info")
                waits = (si or {}).get("on_wait") or []
                if len(waits) > 1:
                    n_split += 1
                    for j, w in enumerate(waits[:-1]):
                        new.append({
                            "debug": ins.get("debug", 0),
                            "engine": ins["engine"],
                            "ins": [], "outs": [],
                            "name": f"{ins['name']}w{j}",
                            "opcode": "NoOp",
                            "sync_info": {"on_update": [], "on_wait": [w]},
                        })
                    si["on_wait"] = [waits[-1]]
                new.append(ins)
            blk["instructions"] = new
    return json.dumps(m).encode()


def build_kernel():
    nc = bass.Bass("TRN2", target_bir_lowering=False, debug=False)
    xT_d = nc.dram_tensor("xT", [D, S], BF16, kind="ExternalInput").ap()
    wqkv_d = nc.dram_tensor("wqkv", [D, 3 * DKC], BF16, kind="ExternalInput").ap()
    bqkv_d = nc.dram_tensor("bqkv", [3 * DKC], F32, kind="ExternalInput").ap()
    wproj_d = nc.dram_tensor("wproj", [DKC, D], BF16, kind="ExternalInput").ap()
    out_d = nc.dram_tensor("out", [S, D], F32, kind="ExternalOutput").ap()

    with tile.TileContext(nc) as tc:
        with ExitStack() as ctx:
            _body(ctx, tc, xT_d, wqkv_d, bqkv_d, wproj_d, out_d)
    orig_to_json = nc.to_json_bytes
    nc.to_json_bytes = lambda: _split_multi_waits(orig_to_json())
    return nc


def _body(ctx, tc, xT_d, wqkv_d, bqkv_d, wproj_d, out_d):
    nc = tc.nc
    ADD = mybir.AluOpType.add
    MULT = mybir.AluOpType.mult
    EXP = mybir.ActivationFunctionType.Exp

    consts = ctx.enter_context(tc.tile_pool(name="consts", bufs=1))
    big = ctx.enter_context(tc.tile_pool(name="big", bufs=1))
    # PSUM: exactly 8 banks: scores 2x[128,1024] + acc 2x[128,512] + misc 2
    scP = ctx.enter_context(tc.tile_pool(name="scP", bufs=2, space="PSUM"))
    accP = ctx.enter_context(tc.tile_pool(name="accP", bufs=2, space="PSUM"))
    miscP = ctx.enter_context(tc.tile_pool(name="miscP", bufs=2, space="PSUM"))
    expp = ctx.enter_context(tc.tile_pool(name="expp", bufs=8))
    smalls = ctx.enter_context(tc.tile_pool(name="smalls", bufs=2))
    outst = ctx.enter_context(tc.tile_pool(name="outst", bufs=3))

    # --- constants / weights / inputs ---
    ones_row = consts.tile([1, P], BF16)
    nc.vector.memset(ones_row, 1.0)

    # Dummy matmuls: keep the PE busy during the initial DMA so the HAM
    # clock-gate is warm (2.4 GHz) when the real qkv matmuls start.
    for _ in range(24):
        warm = miscP.tile([P, P], F32, name="warm", tag="miscP")
        nc.tensor.matmul(warm, lhsT=ones_row, rhs=ones_row, start=True, stop=True)

    w_sb = []
    xT = []
    for c in range(NDC):
        t = big.tile([P, 3 * DKC], BF16, name=f"w_sb{c}")
        nc.sync.dma_start(out=t, in_=wqkv_d[c * P:(c + 1) * P, :])
        w_sb.append(t)
        t = big.tile([P, S], BF16, name=f"xT{c}")
        nc.sync.dma_start(out=t, in_=xT_d[c * P:(c + 1) * P, :])
        xT.append(t)
    wproj_sb = []
    for t3 in range(3):
        t = big.tile([P, D], BF16, name=f"wproj_sb{t3}")
        nc.sync.dma_start(out=t, in_=wproj_d[t3 * P:(t3 + 1) * P, :])
        wproj_sb.append(t)

    bq_sb = consts.tile([P, 3], F32)
    nc.sync.dma_start(out=bq_sb, in_=bqkv_d[0:DKC].rearrange("(t p) -> p t", p=P))
    bk_sb = consts.tile([P, 3], F32)
    nc.sync.dma_start(out=bk_sb, in_=bqkv_d[DKC:2 * DKC].rearrange("(t p) -> p t", p=P))
    # v-bias is NOT applied on device: softmax weights sum to 1, so
    # attn(v + b_v) = attn(v) + b_v, and b_v @ W_proj is added on the host.

    qT = [big.tile([P, S], BF16, name=f"qT{t}") for t in range(3)]
    kT = [big.tile([P, S], BF16, name=f"kT{t}") for t in range(3)]

    def emit_qk(which, t3, sb):
        # which: 0 = q, 1 = k.  Produces (q|k)T[t3][:, sb*512:(sb+1)*512].
        ps = miscP.tile([P, 512], F32, name="psqk", tag="miscP")
        for c in range(NDC):
            nc.tensor.matmul(
                ps,
                lhsT=(w_sb[c][:, which * DKC + t3 * P:which * DKC + (t3 + 1) * P]),
                rhs=(xT[c][:, sb * 512:(sb + 1) * 512]),
                start=(c == 0), stop=(c == NDC - 1))
        dst = (qT if which == 0 else kT)[t3][:, sb * 512:(sb + 1) * 512]
        bias = (bq_sb if which == 0 else bk_sb)[:, t3:t3 + 1]
        nc.vector.tensor_scalar(out=dst, in0=ps, scalar1=bias, scalar2=None, op0=ADD)

    def emit_v(st, vaug):
        ps = miscP.tile([P, DKC], F32, name="psv", tag="miscP")
        for c in range(NDC):
            nc.tensor.matmul(
                ps,
                lhsT=(xT[c][:, st * P:(st + 1) * P]),
                rhs=(w_sb[c][:, 2 * DKC:3 * DKC]),
                start=(c == 0), stop=(c == NDC - 1))
        nc.vector.tensor_copy(
            out=vaug[:, :, st, 0:64],
            in_=ps.rearrange("p (h e) -> p h e", h=HPC))

    # --- qkv for pair 0 first so attention can start early; v and the
    # remaining pairs' k/q are produced inside the attention blocks ---
    vaug = big.tile([P, HPC, NST, P], BF16, name="vaug")
    nc.vector.memset(vaug[:, :, :, 64:128], 1.0)

    for sb in range(NSQ):
        emit_qk(1, 0, sb)        # kT[0]
    emit_qk(0, 0, 0)             # qT[0] @ sb0

    # --- attention + projection ---
    attnT = [big.tile([P, S], BF16, name=f"attnT{pr}") for pr in range(NPAIR)]

    def emit_av(pr, e, accA, accB, ck):
        nc.tensor.matmul(
            accA, lhsT=(vaug[:, 2 * pr, ck, :]), rhs=(e[:, 0:512]),
            start=(ck == 0), stop=(ck == NST - 1))
        nc.tensor.matmul(
            accB, lhsT=(vaug[:, 2 * pr + 1, ck, :]), rhs=(e[:, 512:1024]),
            start=(ck == 0), stop=(ck == NST - 1))

    AV_LAG = 3   # AV(ck) emitted after scores(ck+AV_LAG): the first AVs of a
    # block (which wait on the previous block's acc-bank release) must not
    # block the scores matmuls feeding the exp stream in the PE FIFO.

    def emit_attention_block(pr, sb, companion=None):
        accA = accP.tile([P, 512], F32, name="accA", tag="accP")
        accB = accP.tile([P, 512], F32, name="accB", tag="accP")
        pend = []
        for ck in range(NST):
            if companion is not None:
                companion(ck)
            sc = scP.tile([P, 1024], F32, name="sc", tag="scP")
            nc.tensor.matmul(
                sc[:, 0:512], lhsT=(kT[pr][0:64, ck * P:(ck + 1) * P]),
                rhs=(qT[pr][0:64, sb * 512:(sb + 1) * 512]),
                start=True, stop=True, tile_position=(0, 0))
            nc.tensor.matmul(
                sc[:, 512:1024], lhsT=(kT[pr][64:128, ck * P:(ck + 1) * P]),
                rhs=(qT[pr][64:128, sb * 512:(sb + 1) * 512]),
                start=True, stop=True, tile_position=(64, 0))
            if len(pend) >= AV_LAG:
                cp, ep = pend.pop(0)
                emit_av(pr, ep, accA, accB, cp)
            e = expp.tile([P, 1024], BF16, name="e", tag="expp")
            nc.scalar.activation(out=e, in_=sc, func=EXP)
            pend.append((ck, e))
        for cp, ep in pend:
            emit_av(pr, ep, accA, accB, cp)

        # Drain acc psum to SBUF with cheap copies (releases the acc banks
        # fast so the next block's AV matmuls don't stall the PE FIFO); the
        # expensive reciprocal runs off the critical path, partition-packed
        # so one op covers both heads.
        vals = smalls.tile([P, 512], F32, name="vals", tag="vals")
        den = smalls.tile([P, 512], F32, name="den", tag="den")
        nc.vector.tensor_copy(out=vals[0:64, :], in_=accA[0:64, :])
        nc.vector.tensor_copy(out=den[0:64, :], in_=accA[64:128, :])
        nc.vector.tensor_copy(out=vals[64:128, :], in_=accB[0:64, :])
        nc.vector.tensor_copy(out=den[64:128, :], in_=accB[64:128, :])
        rec = smalls.tile([P, 512], F32, name="rec", tag="rec")
        nc.vector.reciprocal(out=rec, in_=den)
        nc.vector.tensor_tensor(
            out=attnT[pr][:, sb * 512:(sb + 1) * 512],
            in0=vals, in1=rec, op=MULT)

    ostg_tiles = {}

    def emit_proj_piece(sb, idx, pool=None):
        # idx in 0..7: piece (st, half); DMA the staged tile after half 1
        st = sb * 4 + idx // 2
        half = idx % 2
        if half == 0:
            ostg_tiles[st] = outst.tile([P, D], F32, name="ostg", tag="ostg")
        ostg = ostg_tiles[st]
        if pool is None:
            pp = miscP.tile([P, 384], F32, name="pp", tag="miscP")
        else:
            pp = pool.tile([P, 1024], F32, name="ppw", tag="scP")[:, 0:384]
        for t3 in range(3):
            nc.tensor.matmul(
                pp,
                lhsT=(attnT[t3][:, st * P:(st + 1) * P]),
                rhs=(wproj_sb[t3][:, half * 384:(half + 1) * 384]),
                start=(t3 == 0), stop=(t3 == 2))
        nc.vector.tensor_copy(out=ostg[:, half * 384:(half + 1) * 384], in_=pp)
        if half == 1:
            nc.sync.dma_start(out=out_d[st * P:(st + 1) * P, :], in_=ostg)

    def make_companion(pr, sb):
        # per-ck PE side-work emitted under this block's exp stream:
        #  - sb0: v chunks (block 0) and the next pair's k/q (blocks 0,1)
        #  - q for the next sb (each pair produces its own)
        #  - proj pieces of the previous sb, spread across the three blocks
        def companion(ck):
            if sb == 0:
                if pr == 0:
                    emit_v(ck, vaug)
                if pr < 2:
                    if ck in (1, 5, 9, 13):
                        emit_qk(1, pr + 1, (ck - 1) // 4)
                    elif ck == 14:
                        emit_qk(0, pr + 1, 0)
            if ck == 10 and sb + 1 < NSQ:
                emit_qk(0, pr, sb + 1)
            if sb > 0:
                # proj pieces of the previous sb; the first block's pieces go
                # late so they don't wait on the (slow) normalize chain of the
                # previous block in the PE stream
                base = pr * 3
                cks = (12, 13, 14) if pr == 0 else (3, 7, 12)
                for j, ckp in enumerate(cks):
                    if ck == ckp and base + j < 8:
                        emit_proj_piece(sb - 1, base + j)
        return companion

    for sb in range(NSQ):
        for pr in range(NPAIR):
            emit_attention_block(pr, sb, make_companion(pr, sb))
    # tail pieces, pipelined across the now-free score-psum slots
    for idx in range(8):
        emit_proj_piece(NSQ - 1, idx, pool=(scP if idx % 2 else None))


_NC_CACHE = None


def _get_nc():
    global _NC_CACHE
    if _NC_CACHE is None:
        _NC_CACHE = build_kernel()
    return _NC_CACHE


def make_in_maps(hidden_states, W_attn, b_attn, W_proj, b_proj):
    in_maps = []
    scale = 1.0 / np.sqrt(HD)
    for c in range(8):
        b, g = c // 2, c % 2
        cols = slice(g * DKC, (g + 1) * DKC)
        wq = W_attn[:, 0 * D:1 * D][:, cols] * scale
        wk = W_attn[:, 1 * D:2 * D][:, cols]
        wv = W_attn[:, 2 * D:3 * D][:, cols]
        bq = b_attn[0 * D:1 * D][cols] * scale
        bk = b_attn[1 * D:2 * D][cols]
        bv = b_attn[2 * D:3 * D][cols]
        in_maps.append({
            "xT": np.ascontiguousarray(hidden_states[b].T).astype(ml_dtypes.bfloat16),
            "wqkv": np.ascontiguousarray(
                np.concatenate([wq, wk, wv], axis=1)).astype(ml_dtypes.bfloat16),
            "bqkv": np.ascontiguousarray(
                np.concatenate([bq, bk, bv]), dtype=np.float32),
            "wproj": np.ascontiguousarray(
                W_proj[g * DKC:(g + 1) * DKC, :]).astype(ml_dtypes.bfloat16),
        })
    return in_maps


def run(hidden_states, W_attn, b_attn, W_proj, b_proj, trace=False):
    nc = _get_nc()
    in_maps = make_in_maps(hidden_states, W_attn, b_attn, W_proj, b_proj)
    res = run_bass_kernel_spmd(nc, in_maps, core_ids=list(range(8)), trace=trace)
    # v-bias: attn(v + b_v) = attn(v) + b_v since softmax weights sum to 1,
    # so its projection lands as a constant row added host-side.
    bv_proj = np.asarray(b_attn[2 * D:3 * D], dtype=np.float32) @ np.asarray(
        W_proj, dtype=np.float32)
    out = np.empty((B, S, D), dtype=np.float32)
    for b in range(B):
        out[b] = (res.results[2 * b]["out"] + res.results[2 * b + 1]["out"]
                  + b_proj + bv_proj)
    return out, res


def kernel(hidden_states, W_attn, b_attn, W_proj, b_proj):
    hidden_states = np.asarray(hidden_states, dtype=np.float32)
    W_attn = np.asarray(W_attn, dtype=np.float32)
    b_attn = np.asarray(b_attn, dtype=np.float32)
    W_proj = np.asarray(W_proj, dtype=np.float32)
    b_proj = np.asarray(b_proj, dtype=np.float32)
    out, _ = run(hidden_states, W_attn, b_attn, W_proj, b_proj, trace=False)
    return out
